# revision 2
# baseline (speedup 1.0000x reference)
"""Bass/Tile program for nn_ConnectedComponentsModule on one NeuronCore.

64 samples/core. Sample-half grid layout [128 partitions = h*64+s,
34*66 free] with guard ring; CCL via row scans (tensor_tensor_scan) +
6-direction Gauss-Seidel neighbor-min; sizes via one-hot histogram
matmuls; top-20 via max8/match_replace on a composite that packs
(size, root index, color) into one exact f32 integer; bboxes via
occupancy matmuls against static position one-hots; small MLP on
1280 object slots.
"""
import numpy as np
import ml_dtypes
import concourse.bass as bass
import concourse.mybir as mybir
from concourse.mybir import AluOpType as op
from concourse.tile import TileContext

F32 = mybir.dt.float32
BF16 = mybir.dt.bfloat16
U8 = mybir.dt.uint8
I32 = mybir.dt.int32
ACT = mybir.ActivationFunctionType

GN, GRR = 66, 34
FREE = GN * GRR            # 2244
VOFF = GN + 1              # 67: first real cell
VLEN = 2110                # vstep span [67, 2177)
RV = [[GN, 32], [1, 64]]   # real-cell view dims (3D)
CV = [[64, 32], [1, 64]]   # compact view dims matching RV shape
INF = 4096.0
BIG = 65536.0
NITER = 10
K = 20
NS = 64
NB = 10                    # obj blocks (k-major: o = k*64+s)


def fap(t, off, dims):
    a = t[:]
    return bass.AP(a.tensor, a.offset + off, [a.ap[0]] + [list(d) for d in dims])


def rev_ap(a, n):
    return bass.AP(a.tensor, a.offset + n - 1, [a.ap[0], [-1, n]])


def emit_mod(nc, out_ap, in_ap, m, itile, ftile):
    """out = in mod m (m a power of two, in a nonneg integer-valued f32).

    Integer-exact on both CoreSim (trunc convert) and HW (round convert):
    the f32->i32 convert is exact because the value is an integer."""
    from concourse.mybir import AluOpType as op
    nc.vector.tensor_copy(itile, in_ap)
    nc.vector.tensor_scalar(itile, itile, int(m) - 1, None, op.bitwise_and)
    nc.vector.tensor_copy(out_ap, itile)


CSHAPES = {
    "c_flatm": ([128, FREE], F32), "c_iota64": ([128, 64], BF16),
    "c_idn16": ([128, 128], BF16), "c_idnf": ([128, 128], F32),
    "c_posoh": ([128, 32 * 128], BF16), "c_wbb": ([128, 4], BF16),
    "c_revc": ([128, 2048], F32), "c_iota10": ([128, 10], F32),
    "c_w1": ([4, 32], F32), "c_b1": ([128, 32], F32),
    "c_g1": ([128, 32], F32), "c_bb1": ([128, 32], F32),
    "c_w2": ([32, 64], F32), "c_b2": ([128, 64], F32),
    "c_g2": ([128, 64], F32), "c_bb2": ([128, 64], F32),
    "c_cemb": ([10, 32], F32), "c_szw": ([128, 32], F32),
    "c_szb": ([128, 32], F32), "c_szg": ([128, 32], F32),
    "c_szbb": ([128, 32], F32), "c_wc": ([128, 128], F32),
    "c_bc": ([128, 128], F32), "c_gc": ([128, 128], F32),
    "c_bbc": ([128, 128], F32),
}


def build_consts(params):
    c = {}
    flat = np.zeros((128, FREE), np.float32)
    for h in range(2):
        for r in range(32):
            base = (r + 1) * GN + 1
            c0 = (32 * h + r) * 64 - INF
            flat[h * 64:(h + 1) * 64, base:base + 64] = c0 + np.arange(64)
    c["c_flatm"] = flat
    c["c_iota64"] = np.tile(np.arange(64, dtype=np.float32),
                            (128, 1)).astype(ml_dtypes.bfloat16)
    c["c_idn16"] = np.eye(128, dtype=np.float32).astype(ml_dtypes.bfloat16)
    c["c_idnf"] = np.eye(128, dtype=np.float32)
    po = np.zeros((128, 32 * 128), np.float32)
    for h in range(2):
        for cc in range(16):
            q = h * 16 + cc
            for p in range(128):
                po[p, q * 128 + 32 * h + 2 * cc + p // 64] = 1.0
                po[p, q * 128 + 64 + p % 64] = 1.0
    c["c_posoh"] = po.astype(ml_dtypes.bfloat16)
    wbb = np.zeros((128, 4), np.float32)
    wbb[0:64, 0] = 1.0
    wbb[0:64, 1] = np.arange(64)
    wbb[64:128, 2] = 1.0
    wbb[64:128, 3] = np.arange(64)
    c["c_wbb"] = wbb.astype(ml_dtypes.bfloat16)
    rc = np.zeros((128, 2048), np.float32)
    for g in range(2):
        rc[g * 64:(g + 1) * 64, :] = 8191.0 - (g * 2048 + np.arange(2048))
    c["c_revc"] = rc
    c["c_iota10"] = np.tile(np.arange(10, dtype=np.float32), (128, 1))
    rep = lambda v: np.tile(np.asarray(v, np.float32), (128, 1))
    c["c_w1"] = np.asarray(params["obj_w1"], np.float32)
    c["c_b1"] = rep(params["obj_b1"]); c["c_g1"] = rep(params["obj_ln1_g"])
    c["c_bb1"] = rep(params["obj_ln1_b"])
    c["c_w2"] = np.asarray(params["obj_w2"], np.float32)
    c["c_b2"] = rep(params["obj_b2"]); c["c_g2"] = rep(params["obj_ln2_g"])
    c["c_bb2"] = rep(params["obj_ln2_b"])
    c["c_cemb"] = np.asarray(params["color_emb"], np.float32)
    c["c_szw"] = rep(params["size_w"][0]); c["c_szb"] = rep(params["size_b"])
    c["c_szg"] = rep(params["size_ln_g"]); c["c_szbb"] = rep(params["size_ln_b"])
    c["c_wc"] = np.asarray(params["comb_w"], np.float32)
    c["c_bc"] = rep(params["comb_b"]); c["c_gc"] = rep(params["comb_ln_g"])
    c["c_bbc"] = rep(params["comb_ln_b"])
    for n, (shp, dt) in CSHAPES.items():
        want = np.dtype(mybir.dt.np(dt))
        c[n] = np.ascontiguousarray(c[n]).astype(want)
        assert list(c[n].shape) == shp, (n, c[n].shape, shp)
    return c


def build_program(nc: bass.Bass):
    x = nc.dram_tensor("x", [NS, 10, 64, 64], F32, kind="ExternalInput")
    cts = {n: nc.dram_tensor(n, shp, dt, kind="ExternalInput")
           for n, (shp, dt) in CSHAPES.items()}
    feats_o = nc.dram_tensor("feats", [NS, K, 128], F32, kind="ExternalOutput")
    vobj_o = nc.dram_tensor("vobj", [NS, K], U8, kind="ExternalOutput")
    masks_o = nc.dram_tensor("masks", [NS, K, 4096], U8, kind="ExternalOutput")

    with TileContext(nc) as tc:
      with tc.tile_pool(name="consts", bufs=1) as cpool, \
           tc.tile_pool(name="grids", bufs=1) as gpool:

        def cload(name):
            shp, dt = CSHAPES[name]
            t = cpool.tile(shp, dt, name=name + "_t", tag=name)
            nc.sync.dma_start(t[:], cts[name].ap())
            return t

        iota64 = cload("c_iota64")
        idn16 = cload("c_idn16")
        idnf = cload("c_idnf")
        flatm = cload("c_flatm")
        revc = cload("c_revc")

        g_cmap = gpool.tile([128, FREE], F32)
        labs = [gpool.tile([128, FREE], F32, name=f"lab{i}", tag=f"lab{i}") for i in range(3)]
        bestc = gpool.tile([128, 2048], F32)

        # ---------- S0: argmax over channels ----------
        with tc.tile_pool(name="argmax", bufs=3) as xp, \
             tc.tile_pool(name="argmax2", bufs=1) as xp2:
            best = xp2.tile([128, 2048], F32)
            cmp = xp2.tile([128, 2048], F32)
            for ch in range(10):
                xc = xp.tile([128, 2048], F32, tag="xc")
                src = bass.AP(x, ch * 4096, [[2048, 2], [40960, NS], [1, 2048]])
                nc.sync.dma_start(xc[:], src)
                if ch == 0:
                    nc.vector.tensor_copy(best[:], xc[:])
                    nc.vector.memset(bestc[:], 0.0)
                else:
                    nc.vector.tensor_tensor(cmp[:], xc[:], best[:], op.is_gt)
                    nc.vector.tensor_tensor(best[:], best[:], xc[:], op.max)
                    nc.vector.scalar_tensor_tensor(
                        bestc[:], cmp[:], float(ch), bestc[:], op.mult, op.max)

        # ---------- S1: padded grids + penalties ----------
        nc.vector.memset(g_cmap[:], -1.0)
        for t in labs:
            nc.vector.memset(t[:], INF)
        with tc.tile_pool(name="init", bufs=1) as ip:
            m0 = ip.tile([128, 2048], F32)
            t1 = ip.tile([128, 2048], F32)
            nc.vector.tensor_scalar(m0[:], bestc[:], 0.0, None, op.is_gt)
            nc.vector.tensor_scalar(t1[:], bestc[:], 2.0, None, op.add)
            nc.vector.tensor_tensor(t1[:], t1[:], m0[:], op.mult)
            nc.vector.tensor_scalar(t1[:], t1[:], 2.0, None, op.subtract)
            nc.vector.tensor_copy(fap(g_cmap, VOFF, RV), fap(t1, 0, CV))
            nc.vector.tensor_tensor(t1[:], m0[:], fap(flatm, VOFF, RV), op.mult)
            nc.vector.tensor_scalar(t1[:], t1[:], INF, None, op.add)
            nc.vector.tensor_copy(fap(labs[0], VOFF, RV), fap(t1, 0, CV))
        nc.vector.tensor_copy(g_cmap[0:64, 33 * GN:34 * GN],
                              g_cmap[64:128, GN:2 * GN])
        nc.vector.tensor_copy(g_cmap[64:128, 0:GN],
                              g_cmap[0:64, 32 * GN:33 * GN])

        DIRS = (GN + 1, GN, GN - 1, -GN + 1, -GN, -GN - 1)
        with tc.tile_pool(name="pens", bufs=1) as ppool:
            pf = ppool.tile([128, FREE], BF16)
            pb = ppool.tile([128, FREE], BF16)
            nc.vector.memset(pf[:], BIG)
            nc.vector.memset(pb[:], BIG)
            nc.vector.tensor_tensor(fap(pf, 1, [[1, FREE - 1]]),
                                    fap(g_cmap, 1, [[1, FREE - 1]]),
                                    fap(g_cmap, 0, [[1, FREE - 1]]),
                                    op.not_equal)
            nc.vector.tensor_tensor(fap(pb, 0, [[1, FREE - 1]]),
                                    fap(g_cmap, 0, [[1, FREE - 1]]),
                                    fap(g_cmap, 1, [[1, FREE - 1]]),
                                    op.not_equal)
            nc.vector.tensor_scalar(fap(pf, 1, [[1, FREE - 1]]),
                                    fap(pf, 1, [[1, FREE - 1]]),
                                    BIG, None, op.mult)
            nc.vector.tensor_scalar(fap(pb, 0, [[1, FREE - 1]]),
                                    fap(pb, 0, [[1, FREE - 1]]),
                                    BIG, None, op.mult)
            pns = {}
            for d in DIRS:
                pn = ppool.tile([128, VLEN], BF16, tag=f"pn{d}")
                nc.vector.tensor_tensor(pn[:],
                                        fap(g_cmap, VOFF + d, [[1, VLEN]]),
                                        fap(g_cmap, VOFF, [[1, VLEN]]),
                                        op.not_equal)
                pns[d] = pn

            # ---------- S2: CCL superiterations ----------
            with tc.tile_pool(name="ccl", bufs=1) as clp:
                tmp = clp.tile([128, VLEN], F32)
                cur = 0
                for it in range(NITER):
                    ct = labs[cur]
                    nc.vector.tensor_tensor_scan(ct[:], pf[:], ct[:], 1e9,
                                                 op.add, op.min)
                    nc.vector.tensor_tensor_scan(
                        rev_ap(ct[:], FREE), rev_ap(pb[:], FREE),
                        rev_ap(ct[:], FREE), 1e9, op.add, op.min)
                    nc.vector.tensor_copy(ct[0:64, 33 * GN:34 * GN],
                                          ct[64:128, GN:2 * GN])
                    nc.vector.tensor_copy(ct[64:128, 0:GN],
                                          ct[0:64, 32 * GN:33 * GN])
                    a, b = labs[(cur + 1) % 3], labs[(cur + 2) % 3]
                    for t in (a, b):
                        nc.vector.tensor_copy(t[0:64, 33 * GN:34 * GN],
                                              ct[0:64, 33 * GN:34 * GN])
                        nc.vector.tensor_copy(t[64:128, 0:GN],
                                              ct[64:128, 0:GN])
                    seq = [ct, a, b, a, b, a, b]
                    for j, d in enumerate(DIRS):
                        sj, dj = seq[j], seq[j + 1]
                        nc.vector.scalar_tensor_tensor(
                            tmp[:], pns[d][:], BIG,
                            fap(sj, VOFF + d, [[1, VLEN]]), op.mult, op.add)
                        nc.vector.tensor_tensor(
                            fap(dj, VOFF, [[1, VLEN]]),
                            fap(sj, VOFF, [[1, VLEN]]), tmp[:], op.min)
                    cur = labs.index(seq[6])
            lab = labs[cur]

        # ---------- S3: compact hi/lo + transposes ----------
        with tc.tile_pool(name="hilo", bufs=1) as hpool:
            hiT = hpool.tile([128, 16 * 128], BF16)
            loT = hpool.tile([128, 16 * 128], BF16)
            scores2 = hpool.tile([128, 2048], F32)
            with tc.tile_pool(name="hilo_t", bufs=1) as htp, \
                 tc.tile_pool(name="hilo_ps", bufs=2, space="PSUM") as hps:
                labc = htp.tile([128, 2048], F32)
                lo = htp.tile([128, 2048], F32)
                hi16 = htp.tile([128, 2048], BF16)
                lo16 = htp.tile([128, 2048], BF16)
                sci3 = htp.tile([128, 2048], I32)
                scf3 = htp.tile([128, 2048], F32)
                nc.vector.tensor_copy(fap(labc, 0, CV), fap(lab, VOFF, RV))
                emit_mod(nc, lo[:], labc[:], 64.0, sci3[:], scf3[:])
                nc.vector.tensor_copy(lo16[:], lo[:])
                nc.vector.tensor_tensor(lo[:], labc[:], lo[:], op.subtract)
                nc.vector.tensor_scalar(lo[:], lo[:], 1.0 / 64, None, op.mult)
                nc.vector.tensor_copy(hi16[:], lo[:])
                for src16, dstT in ((hi16, hiT), (lo16, loT)):
                    for c in range(16):
                        pt = hps.tile([128, 128], BF16, tag="tp")
                        nc.tensor.transpose(
                            pt[:], src16[:, c * 128:(c + 1) * 128], idn16[:])
                        nc.scalar.copy(dstT[:, c * 128:(c + 1) * 128], pt[:])

            # ---------- S4: one-hot + histogram matmuls ----------
            with tc.tile_pool(name="oh", bufs=4) as ohp, \
                 tc.tile_pool(name="histps", bufs=4, space="PSUM") as hhp, \
                 tc.tile_pool(name="histsb", bufs=4) as hsb:
                for s in range(NS):
                    ps = hhp.tile([64, 64], F32, tag="hist")
                    for h in range(2):
                        sh = h * 64 + s
                        uhi = ohp.tile([128, 1024], BF16, tag="uhi")
                        ulo = ohp.tile([128, 1024], BF16, tag="ulo")
                        nc.vector.tensor_tensor(
                            uhi[:], fap(hiT, sh, [[128, 16], [0, 64]]),
                            fap(iota64, 0, [[0, 16], [1, 64]]), op.is_equal)
                        nc.vector.tensor_tensor(
                            ulo[:], fap(loT, sh, [[128, 16], [0, 64]]),
                            fap(iota64, 0, [[0, 16], [1, 64]]), op.is_equal)
                        for c in range(16):
                            nc.tensor.matmul(
                                ps[:], uhi[:, c * 64:(c + 1) * 64],
                                ulo[:, c * 64:(c + 1) * 64],
                                start=(h == 0 and c == 0),
                                stop=(h == 1 and c == 15))
                    stg = hsb.tile([64, 64], F32, tag="stg")
                    nc.scalar.copy(stg[:], ps[:])
                    nc.sync.dma_start(scores2[s:s + 1, :], stg[0:32, :])
                    nc.sync.dma_start(scores2[64 + s:65 + s, :], stg[32:64, :])

            # ---------- S5: composite top-k ----------
            with tc.tile_pool(name="topk", bufs=1) as tkp:
                comp = tkp.tile([128, 2048], F32)
                m2 = tkp.tile([128, 2048], F32)
                nc.vector.tensor_scalar(m2[:], scores2[:], 2.0, None, op.is_ge)
                nc.vector.tensor_tensor(comp[:], scores2[:], m2[:], op.mult)
                nc.vector.tensor_scalar(comp[:], comp[:], 8192.0, None, op.mult)
                nc.vector.tensor_tensor(comp[:], comp[:], revc[:], op.add)
                nc.vector.tensor_scalar(comp[:], comp[:], 16.0, None, op.mult)
                nc.vector.tensor_tensor(comp[:], comp[:], bestc[:], op.add)
                cand = tkp.tile([128, 24], F32)
                for r in range(3):
                    nc.vector.max(cand[:, r * 8:(r + 1) * 8], comp[:])
                    nc.vector.match_replace(comp[:], cand[:, r * 8:(r + 1) * 8],
                                            comp[:], -1e9)
                cand2 = tkp.tile([64, 48], F32)
                nc.vector.tensor_copy(cand2[:, 0:24], cand[0:64, :])
                nc.vector.tensor_copy(cand2[:, 24:48], cand[64:128, :])
                top24 = gpool.tile([64, 24], F32)
                for r in range(3):
                    nc.vector.max(top24[:, r * 8:(r + 1) * 8], cand2[:])
                    nc.vector.match_replace(cand2[:],
                                            top24[:, r * 8:(r + 1) * 8],
                                            cand2[:], -1e9)

            # decode in sample layout (roots + vobj for masks)
            rootG = gpool.tile([128, K], F32)
            vobjG = gpool.tile([128, K], F32)
            with tc.tile_pool(name="dec", bufs=1) as dcp:
                c16 = dcp.tile([64, 24], F32)
                t16 = dcp.tile([64, 24], F32)
                rmod = dcp.tile([64, 24], F32)
                sco = dcp.tile([64, 24], F32)
                dci = dcp.tile([64, 24], I32)
                dcf = dcp.tile([64, 24], F32)
                emit_mod(nc, c16[:], top24[:], 16.0, dci[:], dcf[:])
                nc.vector.tensor_tensor(t16[:], top24[:], c16[:], op.subtract)
                nc.vector.tensor_scalar(t16[:], t16[:], 1.0 / 16, None, op.mult)
                emit_mod(nc, rmod[:], t16[:], 8192.0, dci[:], dcf[:])
                nc.vector.tensor_tensor(sco[:], t16[:], rmod[:], op.subtract)
                nc.vector.tensor_scalar(sco[:], sco[:], 1.0 / 8192, None,
                                        op.mult)
                nc.vector.tensor_scalar(rmod[:], rmod[:], -1.0, 8191.0,
                                        op.mult, op.add)   # root index
                nc.vector.tensor_scalar(sco[:], sco[:], 2.0, None, op.is_ge)
                nc.vector.tensor_copy(rootG[0:64, :], rmod[:, 0:K])
                nc.vector.tensor_copy(rootG[64:128, :], rmod[:, 0:K])
                nc.vector.tensor_copy(vobjG[0:64, :], sco[:, 0:K])
                nc.vector.tensor_copy(vobjG[64:128, :], sco[:, 0:K])

        # ---------- S6: masks + occupancy matmuls ----------
        posoh = cload("c_posoh")
        wbbt = cload("c_wbb")
        occ_sb = gpool.tile([128, K * 64], BF16)
        with tc.tile_pool(name="mks", bufs=1) as mkp, \
             tc.tile_pool(name="mku", bufs=3) as mkup, \
             tc.tile_pool(name="mtps", bufs=3, space="PSUM") as mtps, \
             tc.tile_pool(name="occps", bufs=1, space="PSUM") as occps, \
             tc.tile_pool(name="mtr", bufs=2) as mtrp:
            occg = [occps.tile([128, 320], F32, name=f"occg{i}", tag=f"occg{i}")
                    for i in range(4)]
            for kg in range(4):
                mbs = []
                for kk in range(5):
                    k = kg * 5 + kk
                    mb = mkp.tile([128, 2048], BF16, name=f"mb{kk}",
                                  tag=f"mb{kk}")
                    nc.vector.tensor_scalar(fap(mb, 0, CV), fap(lab, VOFF, RV),
                                            rootG[:, k:k + 1],
                                            vobjG[:, k:k + 1],
                                            op.is_equal, op.mult)
                    mbs.append(mb)
                    mu8 = mkup.tile([128, 2048], U8, tag="mu8")
                    nc.vector.tensor_scalar(fap(mu8, 0, CV), fap(lab, VOFF, RV),
                                            rootG[:, k:k + 1],
                                            vobjG[:, k:k + 1],
                                            op.is_equal, op.mult)
                    nc.sync.dma_start(
                        bass.AP(masks_o, k * 4096,
                                [[2048, 2], [K * 4096, NS], [1, 2048]]),
                        mu8[:])
                for c in range(16):
                    mtA = mtrp.tile([128, 320], BF16, tag="mtA")
                    mtB = mtrp.tile([128, 320], BF16, tag="mtB")
                    for kk in range(5):
                        pt = mtps.tile([128, 128], BF16, tag="mt")
                        nc.tensor.transpose(
                            pt[:], mbs[kk][:, c * 128:(c + 1) * 128], idn16[:])
                        nc.scalar.copy(mtA[:, kk * 64:(kk + 1) * 64],
                                       pt[:, 0:64])
                        nc.scalar.copy(mtB[:, kk * 64:(kk + 1) * 64],
                                       pt[:, 64:128])
                    for h, mt in ((0, mtA), (1, mtB)):
                        q = h * 16 + c
                        nc.tensor.matmul(
                            occg[kg][:], posoh[:, q * 128:(q + 1) * 128],
                            mt[:], start=(c == 0 and h == 0),
                            stop=(c == 15 and h == 1))
            for kg in range(4):
                nc.vector.tensor_scalar(
                    occ_sb[:, kg * 320:(kg + 1) * 320], occg[kg][:],
                    0.0, None, op.is_gt)

        # ---------- S7: bbox stats + obj-layout rearrange ----------
        with tc.tile_pool(name="obj", bufs=1) as objp:
            bbraw = objp.tile([128, NB * 4], F32)
            with tc.tile_pool(name="bbps", bufs=2, space="PSUM") as bps, \
                 tc.tile_pool(name="bbs", bufs=1) as bbsp:
                bbstS = bbsp.tile([4, K * 64], F32)
                for i, (n0, n1) in enumerate(((0, 512), (512, 1024),
                                              (1024, 1280))):
                    pb_ = bps.tile([4, 512], F32, tag="bbmm")
                    nc.tensor.matmul(pb_[0:4, 0:n1 - n0], wbbt[:],
                                     occ_sb[:, n0:n1], start=True, stop=True)
                    nc.scalar.copy(bbstS[:, n0:n1], pb_[0:4, 0:n1 - n0])
                for bl in range(NB):
                    ptb = bps.tile([128, 4], F32, tag="bbt")
                    nc.tensor.transpose(ptb[:],
                                        bbstS[0:4, bl * 128:(bl + 1) * 128],
                                        idnf[0:4, 0:4])
                    nc.scalar.copy(bbraw[:, bl * 4:(bl + 1) * 4], ptb[:])

            compO = objp.tile([128, NB], F32)
            with tc.tile_pool(name="reps", bufs=2, space="PSUM") as rps, \
                 tc.tile_pool(name="resb", bufs=1) as rsb:
                ptc = rps.tile([24, 64], F32, tag="t24")
                nc.tensor.transpose(ptc[:], top24[:, 0:24], idnf[0:64, 0:64])
                tcomp = rsb.tile([24, 64], F32)
                nc.scalar.copy(tcomp[:], ptc[:])
                for bl in range(NB):
                    nc.sync.dma_start(compO[:, bl:bl + 1],
                                      tcomp[2 * bl:2 * bl + 2, :])

            # decode per-object (k-major layout)
            colO = objp.tile([128, NB], F32)
            scoreO = objp.tile([128, NB], F32)
            vobjO = objp.tile([128, NB], F32)
            osizeO = objp.tile([128, NB], F32)
            tmpO = objp.tile([128, NB], F32)
            oci = objp.tile([128, NB], I32)
            ocf = objp.tile([128, NB], F32)
            emit_mod(nc, colO[:], compO[:], 16.0, oci[:], ocf[:])
            nc.vector.tensor_tensor(tmpO[:], compO[:], colO[:], op.subtract)
            nc.vector.tensor_scalar(tmpO[:], tmpO[:], 1.0 / 16, None, op.mult)
            emit_mod(nc, scoreO[:], tmpO[:], 8192.0, oci[:], ocf[:])
            nc.vector.tensor_tensor(scoreO[:], tmpO[:], scoreO[:], op.subtract)
            nc.vector.tensor_scalar(scoreO[:], scoreO[:], 1.0 / 8192, None,
                                    op.mult)
            nc.vector.tensor_scalar(vobjO[:], scoreO[:], 2.0, None, op.is_ge)
            nc.vector.tensor_scalar(osizeO[:], scoreO[:], 1.0 / 4096, None,
                                    op.mult)
            nc.vector.tensor_tensor(colO[:], colO[:], vobjO[:], op.mult)

            # bbox decode: contiguous row stats then col stats
            bbX = objp.tile([128, NB * 4], F32)
            sc1 = objp.tile([128, NB], F32)
            sc2 = objp.tile([128, NB], F32)
            sci = objp.tile([128, NB], I32)
            for base, fmin, fmax in ((0, 1, 3), (2, 0, 2)):
                nf = fap(bbraw, base, [[4, NB]])
                m1f = fap(bbraw, base + 1, [[4, NB]])
                nc.vector.tensor_scalar(sc1[:], nf, 1.0, None, op.max)
                nc.vector.reciprocal(sc2[:], sc1[:])
                nc.vector.tensor_tensor(sc2[:], sc2[:], m1f, op.mult)
                nc.vector.tensor_scalar(sc2[:], sc2[:], 2.0, 0.4990234375,
                                        op.mult, op.add)
                nc.vector.tensor_copy(sci[:], sc2[:])
                nc.vector.tensor_copy(sc2[:], sci[:])
                nc.vector.tensor_tensor(sc1[:], sc2[:], nf, op.subtract)
                nc.vector.tensor_scalar(sc1[:], sc1[:], 1.0, 0.5 / 64,
                                        op.add, op.mult)
                nc.vector.tensor_tensor(sc1[:], sc1[:], vobjO[:], op.mult)
                nc.vector.tensor_copy(fap(bbX, fmin, [[4, NB]]), sc1[:])
                nc.vector.tensor_tensor(sc1[:], sc2[:], nf, op.add)
                nc.vector.tensor_scalar(sc1[:], sc1[:], -1.0, 0.5 / 64,
                                        op.add, op.mult)
                nc.vector.tensor_tensor(sc1[:], sc1[:], vobjO[:], op.mult)
                nc.vector.tensor_copy(fap(bbX, fmax, [[4, NB]]), sc1[:])

            # ---------- S8: MLP ----------
            w1 = cload("c_w1"); b1 = cload("c_b1")
            g1 = cload("c_g1"); bb1 = cload("c_bb1")
            w2 = cload("c_w2"); b2 = cload("c_b2")
            g2 = cload("c_g2"); bb2 = cload("c_bb2")
            cemb = cload("c_cemb"); szw = cload("c_szw"); szb = cload("c_szb")
            szg = cload("c_szg"); szbb = cload("c_szbb")
            wc = cload("c_wc"); bc = cload("c_bc")
            gc = cload("c_gc"); bbc = cload("c_bbc")
            iota10 = cload("c_iota10")

            def layernorm_relu(z, D, g, bb, out_ap=None):
                mean = objp.tile([128, NB], F32, tag="ln_m")
                var = objp.tile([128, NB], F32, tag="ln_v")
                sq = objp.tile([128, NB * 128], F32, tag="ln_sq")
                z3 = fap(z, 0, [[D, NB], [1, D]])
                sq3 = fap(sq, 0, [[D, NB], [1, D]])
                nc.vector.tensor_reduce(mean[:], z3, mybir.AxisListType.X,
                                        op.add)
                nc.vector.tensor_scalar(mean[:], mean[:], 1.0 / D, None,
                                        op.mult)
                nc.vector.tensor_tensor(z3, z3,
                                        fap(mean, 0, [[1, NB], [0, D]]),
                                        op.subtract)
                nc.vector.tensor_tensor(sq3, z3, z3, op.mult)
                nc.vector.tensor_reduce(var[:], sq3, mybir.AxisListType.X,
                                        op.add)
                nc.vector.tensor_scalar(var[:], var[:], 1.0 / D, 1e-5,
                                        op.mult, op.add)
                nc.scalar.activation(var[:], var[:], ACT.Sqrt)
                nc.vector.reciprocal(var[:], var[:])
                nc.vector.tensor_tensor(z3, z3,
                                        fap(var, 0, [[1, NB], [0, D]]),
                                        op.mult)
                nc.vector.tensor_tensor(z3, z3, fap(g, 0, [[0, NB], [1, D]]),
                                        op.mult)
                nc.vector.tensor_tensor(z3, z3, fap(bb, 0, [[0, NB], [1, D]]),
                                        op.add)
                tgt = out_ap if out_ap is not None else z3
                nc.vector.tensor_scalar(tgt, z3, 0.0, None, op.max)

            comb_in = objp.tile([128, NB * 128], F32)
            with tc.tile_pool(name="mlpps", bufs=2, space="PSUM") as mps, \
                 tc.tile_pool(name="mlpsb", bufs=4) as msb:
                z1 = objp.tile([128, NB * 32], F32)
                for bl in range(NB):
                    ptx = mps.tile([4, 128], F32, tag="tp")
                    nc.tensor.transpose(ptx[:], bbX[:, bl * 4:(bl + 1) * 4],
                                        idnf[:])
                    xt = msb.tile([4, 128], F32, tag="xt1")
                    nc.scalar.copy(xt[:], ptx[:])
                    hp = mps.tile([128, 32], F32, tag="mm")
                    nc.tensor.matmul(hp[:], xt[:], w1[:], start=True, stop=True)
                    nc.scalar.copy(z1[:, bl * 32:(bl + 1) * 32], hp[:])
                z13 = fap(z1, 0, [[32, NB], [1, 32]])
                nc.vector.tensor_tensor(z13, z13,
                                        fap(b1, 0, [[0, NB], [1, 32]]), op.add)
                layernorm_relu(z1, 32, g1, bb1)
                z2 = objp.tile([128, NB * 64], F32)
                for bl in range(NB):
                    ptx = mps.tile([32, 128], F32, tag="tp")
                    nc.tensor.transpose(ptx[:], z1[:, bl * 32:(bl + 1) * 32],
                                        idnf[:])
                    xt = msb.tile([32, 128], F32, tag="xt2")
                    nc.scalar.copy(xt[:], ptx[:])
                    hp = mps.tile([128, 64], F32, tag="mm")
                    nc.tensor.matmul(hp[:], xt[:], w2[:], start=True, stop=True)
                    nc.scalar.copy(z2[:, bl * 64:(bl + 1) * 64], hp[:])
                z23 = fap(z2, 0, [[64, NB], [1, 64]])
                nc.vector.tensor_tensor(z23, z23,
                                        fap(b2, 0, [[0, NB], [1, 64]]), op.add)
                layernorm_relu(z2, 64, g2, bb2,
                               out_ap=fap(comb_in, 0, [[128, NB], [1, 64]]))
                oh10 = objp.tile([128, NB * 10], F32)
                nc.vector.tensor_tensor(fap(oh10, 0, [[10, NB], [1, 10]]),
                                        fap(colO, 0, [[1, NB], [0, 10]]),
                                        fap(iota10, 0, [[0, NB], [1, 10]]),
                                        op.is_equal)
                for bl in range(NB):
                    ptx = mps.tile([10, 128], F32, tag="tp")
                    nc.tensor.transpose(ptx[:], oh10[:, bl * 10:(bl + 1) * 10],
                                        idnf[:])
                    xt = msb.tile([10, 128], F32, tag="xtc")
                    nc.scalar.copy(xt[:], ptx[:])
                    hp = mps.tile([128, 32], F32, tag="mm")
                    nc.tensor.matmul(hp[:], xt[:], cemb[:], start=True,
                                     stop=True)
                    nc.scalar.copy(comb_in[:, bl * 128 + 64:bl * 128 + 96],
                                   hp[:])
                zs = objp.tile([128, NB * 32], F32)
                zs3 = fap(zs, 0, [[32, NB], [1, 32]])
                nc.vector.tensor_tensor(zs3,
                                        fap(osizeO, 0, [[1, NB], [0, 32]]),
                                        fap(szw, 0, [[0, NB], [1, 32]]),
                                        op.mult)
                nc.vector.tensor_tensor(zs3, zs3,
                                        fap(szb, 0, [[0, NB], [1, 32]]),
                                        op.add)
                layernorm_relu(zs, 32, szg, szbb,
                               out_ap=fap(comb_in, 96, [[128, NB], [1, 32]]))
                zc = objp.tile([128, NB * 128], F32)
                for bl in range(NB):
                    ptx = mps.tile([128, 128], F32, tag="tp")
                    nc.tensor.transpose(ptx[:],
                                        comb_in[:, bl * 128:(bl + 1) * 128],
                                        idnf[:])
                    xt = msb.tile([128, 128], F32, tag="xtf")
                    nc.scalar.copy(xt[:], ptx[:])
                    hp = mps.tile([128, 128], F32, tag="mm")
                    nc.tensor.matmul(hp[:], xt[:], wc[:], start=True, stop=True)
                    nc.scalar.copy(zc[:, bl * 128:(bl + 1) * 128], hp[:])
                zc3 = fap(zc, 0, [[128, NB], [1, 128]])
                nc.vector.tensor_tensor(zc3, zc3,
                                        fap(bc, 0, [[0, NB], [1, 128]]),
                                        op.add)
                layernorm_relu(zc, 128, gc, bbc)
                nc.vector.tensor_tensor(zc3, zc3,
                                        fap(vobjO, 0, [[1, NB], [0, 128]]),
                                        op.mult)
                nc.sync.dma_start(
                    bass.AP(feats_o, 0, [[128, 2], [K * 128, NS],
                                         [128 * 2, NB], [1, 128]]),
                    fap(zc, 0, [[128, NB], [1, 128]]))
                vobj8 = objp.tile([128, NB], U8)
                nc.vector.tensor_copy(vobj8[:], vobjO[:])
                nc.sync.dma_start(
                    bass.AP(vobj_o, 0, [[1, 2], [K, NS], [2, NB]]),
                    vobj8[:])
    return nc


# ======================= host driver =======================

def _build_compiled():
    import concourse.bacc as bacc
    nc = bacc.Bacc("TRN2", target_bir_lowering=False, debug=False)
    build_program(nc)
    nc.compile()
    return nc


def kernel(**inputs):
    """Full-input entry: shards batch over 8 NeuronCores, returns full outputs."""
    from concourse.bass_utils import run_bass_kernel_spmd
    x = np.ascontiguousarray(np.asarray(inputs["x"], dtype=np.float32))
    B = x.shape[0]
    n_cores = 8
    per = B // n_cores
    params = {k: np.asarray(v) for k, v in inputs.items() if k != "x"}
    consts = build_consts(params)
    nc = _build_compiled()
    in_maps = []
    for i in range(n_cores):
        m = {"x": np.ascontiguousarray(x[i * per:(i + 1) * per])}
        m.update(consts)
        in_maps.append(m)
    res = run_bass_kernel_spmd(nc, in_maps, list(range(n_cores)))
    feats = np.concatenate([res.results[i]["feats"] for i in range(n_cores)], 0)
    vobj = np.concatenate([res.results[i]["vobj"] for i in range(n_cores)], 0)
    masks = np.concatenate([res.results[i]["masks"] for i in range(n_cores)], 0)
    feats = feats.reshape(B, K, 128).astype(np.float32)
    vobj = vobj.reshape(B, K).astype(bool)
    masks = masks.reshape(B, K, 64, 64).astype(bool)
    return feats, vobj, masks


# revision 4
# speedup vs baseline: 1.1048x; 1.1048x over previous
"""Bass/Tile program for nn_ConnectedComponentsModule on one NeuronCore.

64 samples/core. Sample-half grid layout [128 partitions = h*64+s,
34*66 free] with guard ring; CCL via row scans (tensor_tensor_scan) +
6-direction Gauss-Seidel neighbor-min; sizes via one-hot histogram
matmuls; top-20 via max8/match_replace on a composite that packs
(size, root index, color) into one exact f32 integer; bboxes via
occupancy matmuls against static position one-hots; small MLP on
1280 object slots.
"""
import numpy as np
import ml_dtypes
import concourse.bass as bass
import concourse.mybir as mybir
from concourse.mybir import AluOpType as op
from concourse.tile import TileContext

F32 = mybir.dt.float32
FP8 = mybir.dt.float8e4
BF16 = mybir.dt.bfloat16
U8 = mybir.dt.uint8
I32 = mybir.dt.int32
ACT = mybir.ActivationFunctionType

GN, GRR = 66, 34
FREE = GN * GRR            # 2244
VOFF = GN + 1              # 67: first real cell
VLEN = 2110                # vstep span [67, 2177)
RV = [[GN, 32], [1, 64]]   # real-cell view dims (3D)
CV = [[64, 32], [1, 64]]   # compact view dims matching RV shape
INF = 4096.0
BIG = 65536.0
NITER = 9
K = 20
NS = 64
NB = 10                    # obj blocks (k-major: o = k*64+s)


def fap(t, off, dims):
    a = t[:]
    return bass.AP(a.tensor, a.offset + off, [a.ap[0]] + [list(d) for d in dims])


def rev_ap(a, n):
    return bass.AP(a.tensor, a.offset + n - 1, [a.ap[0], [-1, n]])


def emit_mod(nc, out_ap, in_ap, m, itile, ftile):
    """out = in mod m (m a power of two, in a nonneg integer-valued f32).

    Integer-exact on both CoreSim (trunc convert) and HW (round convert):
    the f32->i32 convert is exact because the value is an integer."""
    from concourse.mybir import AluOpType as op
    nc.vector.tensor_copy(itile, in_ap)
    nc.vector.tensor_scalar(itile, itile, int(m) - 1, None, op.bitwise_and)
    nc.vector.tensor_copy(out_ap, itile)


CSHAPES = {
    "c_flatm": ([128, FREE], F32), "c_iota64": ([128, 64], BF16),
    "c_idn16": ([128, 128], BF16), "c_idnf": ([128, 128], F32),
    "c_revc": ([128, 2048], F32), "c_iota10": ([128, 10], F32),
    "c_rowio": ([128, 32], F32), "c_colio": ([64, 64], F32),
    "c_iotarep": ([128, 1024], BF16),
    "c_w1": ([4, 32], F32), "c_b1": ([128, 32], F32),
    "c_g1": ([128, 32], F32), "c_bb1": ([128, 32], F32),
    "c_w2": ([32, 64], F32), "c_b2": ([128, 64], F32),
    "c_g2": ([128, 64], F32), "c_bb2": ([128, 64], F32),
    "c_cemb": ([10, 32], F32), "c_szw": ([128, 32], F32),
    "c_szb": ([128, 32], F32), "c_szg": ([128, 32], F32),
    "c_szbb": ([128, 32], F32), "c_wc": ([128, 128], F32),
    "c_bc": ([128, 128], F32), "c_gc": ([128, 128], F32),
    "c_bbc": ([128, 128], F32),
}


def build_consts(params):
    c = {}
    flat = np.zeros((128, FREE), np.float32)
    for h in range(2):
        for r in range(32):
            base = (r + 1) * GN + 1
            c0 = (32 * h + r) * 64 - INF
            flat[h * 64:(h + 1) * 64, base:base + 64] = c0 + np.arange(64)
    c["c_flatm"] = flat
    c["c_iota64"] = np.tile(np.arange(64, dtype=np.float32),
                            (128, 1)).astype(ml_dtypes.bfloat16)
    c["c_idn16"] = np.eye(128, dtype=np.float32).astype(ml_dtypes.bfloat16)
    c["c_idnf"] = np.eye(128, dtype=np.float32)
    rio = np.zeros((128, 32), np.float32)
    rio[0:64, :] = np.arange(32)
    rio[64:128, :] = 32 + np.arange(32)
    c["c_rowio"] = rio
    c["c_colio"] = np.tile(np.arange(64, dtype=np.float32), (64, 1))
    c["c_iotarep"] = np.tile(np.arange(64, dtype=np.float32),
                             (128, 16)).astype(ml_dtypes.bfloat16)
    rc = np.zeros((128, 2048), np.float32)
    for g in range(2):
        rc[g * 64:(g + 1) * 64, :] = 8191.0 - (g * 2048 + np.arange(2048))
    c["c_revc"] = rc
    c["c_iota10"] = np.tile(np.arange(10, dtype=np.float32), (128, 1))
    rep = lambda v: np.tile(np.asarray(v, np.float32), (128, 1))
    c["c_w1"] = np.asarray(params["obj_w1"], np.float32)
    c["c_b1"] = rep(params["obj_b1"]); c["c_g1"] = rep(params["obj_ln1_g"])
    c["c_bb1"] = rep(params["obj_ln1_b"])
    c["c_w2"] = np.asarray(params["obj_w2"], np.float32)
    c["c_b2"] = rep(params["obj_b2"]); c["c_g2"] = rep(params["obj_ln2_g"])
    c["c_bb2"] = rep(params["obj_ln2_b"])
    c["c_cemb"] = np.asarray(params["color_emb"], np.float32)
    c["c_szw"] = rep(params["size_w"][0]); c["c_szb"] = rep(params["size_b"])
    c["c_szg"] = rep(params["size_ln_g"]); c["c_szbb"] = rep(params["size_ln_b"])
    c["c_wc"] = np.asarray(params["comb_w"], np.float32)
    c["c_bc"] = rep(params["comb_b"]); c["c_gc"] = rep(params["comb_ln_g"])
    c["c_bbc"] = rep(params["comb_ln_b"])
    for n, (shp, dt) in CSHAPES.items():
        want = np.dtype(mybir.dt.np(dt))
        c[n] = np.ascontiguousarray(c[n]).astype(want)
        assert list(c[n].shape) == shp, (n, c[n].shape, shp)
    return c


def build_program(nc: bass.Bass):
    x = nc.dram_tensor("x", [NS, 10, 64, 64], F32, kind="ExternalInput")
    cts = {n: nc.dram_tensor(n, shp, dt, kind="ExternalInput")
           for n, (shp, dt) in CSHAPES.items()}
    feats_o = nc.dram_tensor("feats", [NS, K, 128], F32, kind="ExternalOutput")
    vobj_o = nc.dram_tensor("vobj", [NS, K], U8, kind="ExternalOutput")
    masks_o = nc.dram_tensor("masks", [NS, K, 4096], U8, kind="ExternalOutput")

    with TileContext(nc) as tc:
      with tc.tile_pool(name="consts", bufs=1) as cpool, \
           tc.tile_pool(name="grids", bufs=1) as gpool:

        def cload(name):
            shp, dt = CSHAPES[name]
            t = cpool.tile(shp, dt, name=name + "_t", tag=name)
            nc.sync.dma_start(t[:], cts[name].ap())
            return t

        iota64 = cload("c_iota64")
        iotarep = cload("c_iotarep")
        idn16 = cload("c_idn16")
        idnf = cload("c_idnf")
        flatm = cload("c_flatm")
        revc = cload("c_revc")

        g_cmap = gpool.tile([128, FREE], F32)
        labs = [gpool.tile([128, FREE], F32, name=f"lab{i}", tag=f"lab{i}") for i in range(3)]
        bestc = gpool.tile([128, 2048], F32)

        # ---------- S0: argmax over channels ----------
        with tc.tile_pool(name="argmax", bufs=3) as xp, \
             tc.tile_pool(name="argmax2", bufs=1) as xp2:
            best = xp2.tile([128, 2048], F32)
            cmp = xp2.tile([128, 2048], F32)
            for ch in range(10):
                xc = xp.tile([128, 2048], F32, tag="xc")
                src = bass.AP(x, ch * 4096, [[2048, 2], [40960, NS], [1, 2048]])
                nc.sync.dma_start(xc[:], src)
                if ch == 0:
                    nc.vector.tensor_copy(best[:], xc[:])
                    nc.vector.memset(bestc[:], 0.0)
                else:
                    nc.vector.tensor_tensor(cmp[:], xc[:], best[:], op.is_gt)
                    nc.vector.tensor_tensor(best[:], best[:], xc[:], op.max)
                    nc.vector.scalar_tensor_tensor(
                        bestc[:], cmp[:], float(ch), bestc[:], op.mult, op.max)

        # ---------- S1: padded grids + penalties ----------
        nc.vector.memset(g_cmap[:], -1.0)
        for t in labs:
            nc.vector.memset(t[:], INF)
        with tc.tile_pool(name="init", bufs=1) as ip:
            m0 = ip.tile([128, 2048], F32)
            t1 = ip.tile([128, 2048], F32)
            nc.vector.tensor_scalar(m0[:], bestc[:], 0.0, None, op.is_gt)
            nc.vector.tensor_scalar(t1[:], bestc[:], 2.0, None, op.add)
            nc.vector.tensor_tensor(t1[:], t1[:], m0[:], op.mult)
            nc.vector.tensor_scalar(t1[:], t1[:], 2.0, None, op.subtract)
            nc.vector.tensor_copy(fap(g_cmap, VOFF, RV), fap(t1, 0, CV))
            nc.vector.tensor_tensor(t1[:], m0[:], fap(flatm, VOFF, RV), op.mult)
            nc.vector.tensor_scalar(t1[:], t1[:], INF, None, op.add)
            nc.vector.tensor_copy(fap(labs[0], VOFF, RV), fap(t1, 0, CV))
        nc.vector.tensor_copy(g_cmap[0:64, 33 * GN:34 * GN],
                              g_cmap[64:128, GN:2 * GN])
        nc.vector.tensor_copy(g_cmap[64:128, 0:GN],
                              g_cmap[0:64, 32 * GN:33 * GN])

        DIRS = (GN + 1, GN, GN - 1, -GN + 1, -GN, -GN - 1)
        with tc.tile_pool(name="pens", bufs=1) as ppool:
            pf = ppool.tile([128, FREE], BF16)
            pb = ppool.tile([128, FREE], BF16)
            nc.vector.memset(pf[:], BIG)
            nc.vector.memset(pb[:], BIG)
            nc.vector.tensor_tensor(fap(pf, 1, [[1, FREE - 1]]),
                                    fap(g_cmap, 1, [[1, FREE - 1]]),
                                    fap(g_cmap, 0, [[1, FREE - 1]]),
                                    op.not_equal)
            nc.vector.tensor_tensor(fap(pb, 0, [[1, FREE - 1]]),
                                    fap(g_cmap, 0, [[1, FREE - 1]]),
                                    fap(g_cmap, 1, [[1, FREE - 1]]),
                                    op.not_equal)
            nc.vector.tensor_scalar(fap(pf, 1, [[1, FREE - 1]]),
                                    fap(pf, 1, [[1, FREE - 1]]),
                                    BIG, None, op.mult)
            nc.vector.tensor_scalar(fap(pb, 0, [[1, FREE - 1]]),
                                    fap(pb, 0, [[1, FREE - 1]]),
                                    BIG, None, op.mult)
            pns = {}
            for d in DIRS:
                pn = ppool.tile([128, VLEN], BF16, tag=f"pn{d}", name=f"pn{d}")
                nc.vector.tensor_tensor(pn[:],
                                        fap(g_cmap, VOFF + d, [[1, VLEN]]),
                                        fap(g_cmap, VOFF, [[1, VLEN]]),
                                        op.not_equal)
                nc.vector.tensor_scalar(pn[:], pn[:], BIG, None, op.mult)
                pns[d] = pn

            # ---------- S2: CCL superiterations ----------
            with tc.tile_pool(name="ccl", bufs=1) as clp:
                tmp = clp.tile([128, VLEN], F32)
                cur = 0
                for it in range(NITER):
                    ct = labs[cur]
                    nc.vector.tensor_tensor_scan(ct[:], pf[:], ct[:], 1e9,
                                                 op.add, op.min)
                    nc.vector.tensor_tensor_scan(
                        rev_ap(ct[:], FREE), rev_ap(pb[:], FREE),
                        rev_ap(ct[:], FREE), 1e9, op.add, op.min)
                    nc.vector.tensor_copy(ct[0:64, 33 * GN:34 * GN],
                                          ct[64:128, GN:2 * GN])
                    nc.vector.tensor_copy(ct[64:128, 0:GN],
                                          ct[0:64, 32 * GN:33 * GN])
                    a, b = labs[(cur + 1) % 3], labs[(cur + 2) % 3]
                    for t in (a, b):
                        nc.vector.tensor_copy(t[0:64, 33 * GN:34 * GN],
                                              ct[0:64, 33 * GN:34 * GN])
                        nc.vector.tensor_copy(t[64:128, 0:GN],
                                              ct[64:128, 0:GN])
                    seq = [ct, a, b, a, b, a, b]
                    for j, d in enumerate(DIRS):
                        sj, dj = seq[j], seq[j + 1]
                        nc.vector.tensor_tensor(
                            tmp[:], pns[d][:],
                            fap(sj, VOFF + d, [[1, VLEN]]), op.add)
                        nc.vector.tensor_tensor(
                            fap(dj, VOFF, [[1, VLEN]]),
                            fap(sj, VOFF, [[1, VLEN]]), tmp[:], op.min)
                    cur = labs.index(seq[6])
            lab = labs[cur]

        # ---------- S3: compact hi/lo + transposes ----------
        with tc.tile_pool(name="hilo", bufs=1) as hpool:
            hiT = hpool.tile([128, 16 * 128], BF16)
            loT = hpool.tile([128, 16 * 128], BF16)
            scores2 = hpool.tile([128, 2048], F32)
            with tc.tile_pool(name="hilo_t", bufs=1) as htp, \
                 tc.tile_pool(name="hilo_ps", bufs=2, space="PSUM") as hps:
                labc = htp.tile([128, 2048], F32)
                lo = htp.tile([128, 2048], F32)
                hi16 = htp.tile([128, 2048], BF16)
                lo16 = htp.tile([128, 2048], BF16)
                sci3 = htp.tile([128, 2048], I32)
                scf3 = htp.tile([128, 2048], F32)
                nc.vector.tensor_copy(fap(labc, 0, CV), fap(lab, VOFF, RV))
                emit_mod(nc, lo[:], labc[:], 64.0, sci3[:], scf3[:])
                nc.vector.tensor_copy(lo16[:], lo[:])
                nc.vector.tensor_tensor(lo[:], labc[:], lo[:], op.subtract)
                nc.vector.tensor_scalar(lo[:], lo[:], 1.0 / 64, None, op.mult)
                nc.vector.tensor_copy(hi16[:], lo[:])
                for src16, dstT in ((hi16, hiT), (lo16, loT)):
                    for c in range(16):
                        pt = hps.tile([128, 128], BF16, tag="tp")
                        nc.tensor.transpose(
                            pt[:], src16[:, c * 128:(c + 1) * 128], idn16[:])
                        nc.scalar.copy(dstT[:, c * 128:(c + 1) * 128], pt[:])

            # ---------- S4: one-hot + histogram matmuls ----------
            with tc.tile_pool(name="oh", bufs=4) as ohp, \
                 tc.tile_pool(name="histps", bufs=4, space="PSUM") as hhp, \
                 tc.tile_pool(name="histsb", bufs=4) as hsb:
                for s in range(NS):
                    ps = hhp.tile([64, 64], F32, tag="hist")
                    for h in range(2):
                        sh = h * 64 + s
                        uhi = ohp.tile([128, 1024], FP8, tag="uhi")
                        ulo = ohp.tile([128, 1024], FP8, tag="ulo")
                        nc.vector.tensor_tensor(
                            uhi[:], fap(hiT, sh, [[128, 16], [0, 64]]),
                            iotarep[:], op.is_equal)
                        nc.vector.tensor_tensor(
                            ulo[:], fap(loT, sh, [[128, 16], [0, 64]]),
                            iotarep[:], op.is_equal)
                        for c2 in range(8):
                            nc.tensor.matmul(
                                ps[:],
                                fap(uhi, c2 * 128, [[64, 2], [1, 64]]),
                                fap(ulo, c2 * 128, [[64, 2], [1, 64]]),
                                start=(h == 0 and c2 == 0),
                                stop=(h == 1 and c2 == 7),
                                perf_mode=mybir.MatmulPerfMode.DoubleRow)
                    stg = hsb.tile([64, 64], F32, tag="stg")
                    nc.scalar.copy(stg[:], ps[:])
                    nc.sync.dma_start(scores2[s:s + 1, :], stg[0:32, :])
                    nc.sync.dma_start(scores2[64 + s:65 + s, :], stg[32:64, :])

            # ---------- S5: composite top-k ----------
            with tc.tile_pool(name="topk", bufs=1) as tkp:
                comp = tkp.tile([128, 2048], F32)
                m2 = tkp.tile([128, 2048], F32)
                nc.vector.tensor_scalar(m2[:], scores2[:], 2.0, None, op.is_ge)
                nc.vector.tensor_tensor(m2[:], scores2[:], m2[:], op.mult)
                nc.vector.scalar_tensor_tensor(comp[:], m2[:], 8192.0,
                                               revc[:], op.mult, op.add)
                nc.vector.scalar_tensor_tensor(comp[:], comp[:], 16.0,
                                               bestc[:], op.mult, op.add)
                cand = tkp.tile([128, 24], F32)
                for r in range(3):
                    nc.vector.max(cand[:, r * 8:(r + 1) * 8], comp[:])
                    nc.vector.match_replace(comp[:], cand[:, r * 8:(r + 1) * 8],
                                            comp[:], -1e9)
                cand2 = tkp.tile([64, 48], F32)
                nc.vector.tensor_copy(cand2[:, 0:24], cand[0:64, :])
                nc.vector.tensor_copy(cand2[:, 24:48], cand[64:128, :])
                top24 = gpool.tile([64, 24], F32)
                for r in range(3):
                    nc.vector.max(top24[:, r * 8:(r + 1) * 8], cand2[:])
                    nc.vector.match_replace(cand2[:],
                                            top24[:, r * 8:(r + 1) * 8],
                                            cand2[:], -1e9)

            # decode in sample layout (roots + vobj for masks)
            rootG = gpool.tile([128, K], F32)
            vobjG = gpool.tile([128, K], F32)
            with tc.tile_pool(name="dec", bufs=1) as dcp:
                c16 = dcp.tile([64, 24], F32)
                t16 = dcp.tile([64, 24], F32)
                rmod = dcp.tile([64, 24], F32)
                sco = dcp.tile([64, 24], F32)
                dci = dcp.tile([64, 24], I32)
                dcf = dcp.tile([64, 24], F32)
                emit_mod(nc, c16[:], top24[:], 16.0, dci[:], dcf[:])
                nc.vector.tensor_tensor(t16[:], top24[:], c16[:], op.subtract)
                nc.vector.tensor_scalar(t16[:], t16[:], 1.0 / 16, None, op.mult)
                emit_mod(nc, rmod[:], t16[:], 8192.0, dci[:], dcf[:])
                nc.vector.tensor_tensor(sco[:], t16[:], rmod[:], op.subtract)
                nc.vector.tensor_scalar(sco[:], sco[:], 1.0 / 8192, None,
                                        op.mult)
                nc.vector.tensor_scalar(rmod[:], rmod[:], -1.0, 8191.0,
                                        op.mult, op.add)   # root index
                nc.vector.tensor_scalar(sco[:], sco[:], 2.0, None, op.is_ge)
                nc.vector.tensor_copy(rootG[0:64, :], rmod[:, 0:K])
                nc.vector.tensor_copy(rootG[64:128, :], rmod[:, 0:K])
                nc.vector.tensor_copy(vobjG[0:64, :], sco[:, 0:K])
                nc.vector.tensor_copy(vobjG[64:128, :], sco[:, 0:K])

        # ---------- S6: masks + row/col occupancy via DVE reduces ----------
        rowio = cload("c_rowio")
        colio = cload("c_colio")
        ffld = gpool.tile([64, 4 * K], F32)   # cols f*K+k: n_r, m1_r, n_c, m1_c
        with tc.tile_pool(name="mks", bufs=3) as mkp, \
             tc.tile_pool(name="mku", bufs=3) as mkup, \
             tc.tile_pool(name="mkr", bufs=2) as mkr:
            for k in range(K):
                mb = mkp.tile([128, 2048], BF16, tag="mb")
                nc.vector.tensor_scalar(fap(mb, 0, CV), fap(lab, VOFF, RV),
                                        rootG[:, k:k + 1],
                                        vobjG[:, k:k + 1],
                                        op.is_equal, op.mult)
                mu8 = mkup.tile([128, 2048], U8, tag="mu8")
                nc.scalar.copy(mu8[:], mb[:])
                nc.sync.dma_start(
                    bass.AP(masks_o, k * 4096,
                            [[2048, 2], [K * 4096, NS], [1, 2048]]),
                    mu8[:])
                rowo = mkr.tile([128, 32], F32, tag="rowo")
                colo = mkr.tile([128, 64], F32, tag="colo")
                t1 = mkr.tile([128, 1], F32, tag="t1")
                t1b = mkr.tile([64, 1], F32, tag="t1b")
                t2 = mkr.tile([128, 32], F32, tag="t2")
                t3 = mkr.tile([64, 64], F32, tag="t3")
                t3b = mkr.tile([64, 64], F32, tag="t3b")
                nc.vector.tensor_reduce(rowo[:], fap(mb, 0, [[64, 32], [1, 64]]),
                                        mybir.AxisListType.X, op.add)
                nc.vector.tensor_reduce(colo[:], fap(mb, 0, [[1, 64], [64, 32]]),
                                        mybir.AxisListType.X, op.add)
                nc.vector.tensor_scalar(rowo[:], rowo[:], 0.0, None, op.is_gt)
                nc.vector.tensor_scalar(colo[:], colo[:], 0.0, None, op.is_gt)
                nc.vector.tensor_reduce(t1[:], rowo[:],
                                        mybir.AxisListType.X, op.add)
                nc.vector.tensor_copy(t1b[:], t1[64:128, :])
                nc.vector.tensor_tensor(ffld[:, 0 * K + k:0 * K + k + 1],
                                        t1[0:64, :], t1b[:], op.add)
                nc.vector.tensor_tensor(t2[:], rowo[:], rowio[:], op.mult)
                nc.vector.tensor_reduce(t1[:], t2[:],
                                        mybir.AxisListType.X, op.add)
                nc.vector.tensor_copy(t1b[:], t1[64:128, :])
                nc.vector.tensor_tensor(ffld[:, 1 * K + k:1 * K + k + 1],
                                        t1[0:64, :], t1b[:], op.add)
                nc.vector.tensor_copy(t3b[:], colo[64:128, :])
                nc.vector.tensor_tensor(t3[:], colo[0:64, :], t3b[:],
                                        op.max)
                nc.vector.tensor_reduce(ffld[:, 2 * K + k:2 * K + k + 1],
                                        t3[:], mybir.AxisListType.X, op.add)
                nc.vector.tensor_tensor(t3[:], t3[:], colio[:], op.mult)
                nc.vector.tensor_reduce(ffld[:, 3 * K + k:3 * K + k + 1],
                                        t3[:], mybir.AxisListType.X, op.add)

        # ---------- S7: obj-layout rearrange ----------
        with tc.tile_pool(name="obj", bufs=1) as objp:
            bbraw = objp.tile([128, NB * 4], F32)
            with tc.tile_pool(name="fldps", bufs=2, space="PSUM") as fps, \
                 tc.tile_pool(name="flds", bufs=1) as fsb:
                ptf = fps.tile([4 * K, 64], F32, tag="tf")
                nc.tensor.transpose(ptf[:], ffld[:, 0:4 * K], idnf[0:64, 0:64])
                trf = fsb.tile([4 * K, 64], F32)
                nc.scalar.copy(trf[:], ptf[:])
                for bl in range(NB):
                    for f in range(4):
                        nc.sync.dma_start(
                            bbraw[:, bl * 4 + f:bl * 4 + f + 1],
                            trf[f * K + 2 * bl:f * K + 2 * bl + 2, :])

            compO = objp.tile([128, NB], F32)
            with tc.tile_pool(name="reps", bufs=2, space="PSUM") as rps, \
                 tc.tile_pool(name="resb", bufs=1) as rsb:
                ptc = rps.tile([24, 64], F32, tag="t24")
                nc.tensor.transpose(ptc[:], top24[:, 0:24], idnf[0:64, 0:64])
                tcomp = rsb.tile([24, 64], F32)
                nc.scalar.copy(tcomp[:], ptc[:])
                for bl in range(NB):
                    nc.sync.dma_start(compO[:, bl:bl + 1],
                                      tcomp[2 * bl:2 * bl + 2, :])

            # decode per-object (k-major layout)
            colO = objp.tile([128, NB], F32)
            scoreO = objp.tile([128, NB], F32)
            vobjO = objp.tile([128, NB], F32)
            osizeO = objp.tile([128, NB], F32)
            tmpO = objp.tile([128, NB], F32)
            oci = objp.tile([128, NB], I32)
            ocf = objp.tile([128, NB], F32)
            emit_mod(nc, colO[:], compO[:], 16.0, oci[:], ocf[:])
            nc.vector.tensor_tensor(tmpO[:], compO[:], colO[:], op.subtract)
            nc.vector.tensor_scalar(tmpO[:], tmpO[:], 1.0 / 16, None, op.mult)
            emit_mod(nc, scoreO[:], tmpO[:], 8192.0, oci[:], ocf[:])
            nc.vector.tensor_tensor(scoreO[:], tmpO[:], scoreO[:], op.subtract)
            nc.vector.tensor_scalar(scoreO[:], scoreO[:], 1.0 / 8192, None,
                                    op.mult)
            nc.vector.tensor_scalar(vobjO[:], scoreO[:], 2.0, None, op.is_ge)
            nc.vector.tensor_scalar(osizeO[:], scoreO[:], 1.0 / 4096, None,
                                    op.mult)
            nc.vector.tensor_tensor(colO[:], colO[:], vobjO[:], op.mult)

            # bbox decode: contiguous row stats then col stats
            bbX = objp.tile([128, NB * 4], F32)
            sc1 = objp.tile([128, NB], F32)
            sc2 = objp.tile([128, NB], F32)
            sci = objp.tile([128, NB], I32)
            for base, fmin, fmax in ((0, 1, 3), (2, 0, 2)):
                nf = fap(bbraw, base, [[4, NB]])
                m1f = fap(bbraw, base + 1, [[4, NB]])
                nc.vector.tensor_scalar(sc1[:], nf, 1.0, None, op.max)
                nc.vector.reciprocal(sc2[:], sc1[:])
                nc.vector.tensor_tensor(sc2[:], sc2[:], m1f, op.mult)
                nc.vector.tensor_scalar(sc2[:], sc2[:], 2.0, 0.4990234375,
                                        op.mult, op.add)
                nc.vector.tensor_copy(sci[:], sc2[:])
                nc.vector.tensor_copy(sc2[:], sci[:])
                nc.vector.tensor_tensor(sc1[:], sc2[:], nf, op.subtract)
                nc.vector.tensor_scalar(sc1[:], sc1[:], 1.0, 0.5 / 64,
                                        op.add, op.mult)
                nc.vector.tensor_tensor(sc1[:], sc1[:], vobjO[:], op.mult)
                nc.vector.tensor_copy(fap(bbX, fmin, [[4, NB]]), sc1[:])
                nc.vector.tensor_tensor(sc1[:], sc2[:], nf, op.add)
                nc.vector.tensor_scalar(sc1[:], sc1[:], -1.0, 0.5 / 64,
                                        op.add, op.mult)
                nc.vector.tensor_tensor(sc1[:], sc1[:], vobjO[:], op.mult)
                nc.vector.tensor_copy(fap(bbX, fmax, [[4, NB]]), sc1[:])

            # ---------- S8: MLP ----------
            w1 = cload("c_w1"); b1 = cload("c_b1")
            g1 = cload("c_g1"); bb1 = cload("c_bb1")
            w2 = cload("c_w2"); b2 = cload("c_b2")
            g2 = cload("c_g2"); bb2 = cload("c_bb2")
            cemb = cload("c_cemb"); szw = cload("c_szw"); szb = cload("c_szb")
            szg = cload("c_szg"); szbb = cload("c_szbb")
            wc = cload("c_wc"); bc = cload("c_bc")
            gc = cload("c_gc"); bbc = cload("c_bbc")
            iota10 = cload("c_iota10")

            def layernorm_relu(z, D, g, bb, out_ap=None):
                mean = objp.tile([128, NB], F32, tag="ln_m")
                var = objp.tile([128, NB], F32, tag="ln_v")
                sq = objp.tile([128, NB * 128], F32, tag="ln_sq")
                z3 = fap(z, 0, [[D, NB], [1, D]])
                sq3 = fap(sq, 0, [[D, NB], [1, D]])
                nc.vector.tensor_reduce(mean[:], z3, mybir.AxisListType.X,
                                        op.add)
                nc.vector.tensor_scalar(mean[:], mean[:], 1.0 / D, None,
                                        op.mult)
                nc.vector.tensor_tensor(z3, z3,
                                        fap(mean, 0, [[1, NB], [0, D]]),
                                        op.subtract)
                nc.vector.tensor_tensor(sq3, z3, z3, op.mult)
                nc.vector.tensor_reduce(var[:], sq3, mybir.AxisListType.X,
                                        op.add)
                nc.vector.tensor_scalar(var[:], var[:], 1.0 / D, 1e-5,
                                        op.mult, op.add)
                nc.scalar.activation(var[:], var[:], ACT.Sqrt)
                nc.vector.reciprocal(var[:], var[:])
                nc.vector.tensor_tensor(z3, z3,
                                        fap(var, 0, [[1, NB], [0, D]]),
                                        op.mult)
                nc.vector.tensor_tensor(z3, z3, fap(g, 0, [[0, NB], [1, D]]),
                                        op.mult)
                nc.vector.tensor_tensor(z3, z3, fap(bb, 0, [[0, NB], [1, D]]),
                                        op.add)
                tgt = out_ap if out_ap is not None else z3
                nc.vector.tensor_scalar(tgt, z3, 0.0, None, op.max)

            comb_in = objp.tile([128, NB * 128], F32)
            with tc.tile_pool(name="mlpps", bufs=2, space="PSUM") as mps, \
                 tc.tile_pool(name="mlpsb", bufs=4) as msb:
                z1 = objp.tile([128, NB * 32], F32)
                for bl in range(NB):
                    ptx = mps.tile([4, 128], F32, tag="tp")
                    nc.tensor.transpose(ptx[:], bbX[:, bl * 4:(bl + 1) * 4],
                                        idnf[:])
                    xt = msb.tile([4, 128], F32, tag="xt1")
                    nc.scalar.copy(xt[:], ptx[:])
                    hp = mps.tile([128, 32], F32, tag="mm")
                    nc.tensor.matmul(hp[:], xt[:], w1[:], start=True, stop=True)
                    nc.scalar.copy(z1[:, bl * 32:(bl + 1) * 32], hp[:])
                z13 = fap(z1, 0, [[32, NB], [1, 32]])
                nc.vector.tensor_tensor(z13, z13,
                                        fap(b1, 0, [[0, NB], [1, 32]]), op.add)
                layernorm_relu(z1, 32, g1, bb1)
                z2 = objp.tile([128, NB * 64], F32)
                for bl in range(NB):
                    ptx = mps.tile([32, 128], F32, tag="tp")
                    nc.tensor.transpose(ptx[:], z1[:, bl * 32:(bl + 1) * 32],
                                        idnf[:])
                    xt = msb.tile([32, 128], F32, tag="xt2")
                    nc.scalar.copy(xt[:], ptx[:])
                    hp = mps.tile([128, 64], F32, tag="mm")
                    nc.tensor.matmul(hp[:], xt[:], w2[:], start=True, stop=True)
                    nc.scalar.copy(z2[:, bl * 64:(bl + 1) * 64], hp[:])
                z23 = fap(z2, 0, [[64, NB], [1, 64]])
                nc.vector.tensor_tensor(z23, z23,
                                        fap(b2, 0, [[0, NB], [1, 64]]), op.add)
                layernorm_relu(z2, 64, g2, bb2,
                               out_ap=fap(comb_in, 0, [[128, NB], [1, 64]]))
                oh10 = objp.tile([128, NB * 10], F32)
                nc.vector.tensor_tensor(fap(oh10, 0, [[10, NB], [1, 10]]),
                                        fap(colO, 0, [[1, NB], [0, 10]]),
                                        fap(iota10, 0, [[0, NB], [1, 10]]),
                                        op.is_equal)
                for bl in range(NB):
                    ptx = mps.tile([10, 128], F32, tag="tp")
                    nc.tensor.transpose(ptx[:], oh10[:, bl * 10:(bl + 1) * 10],
                                        idnf[:])
                    xt = msb.tile([10, 128], F32, tag="xtc")
                    nc.scalar.copy(xt[:], ptx[:])
                    hp = mps.tile([128, 32], F32, tag="mm")
                    nc.tensor.matmul(hp[:], xt[:], cemb[:], start=True,
                                     stop=True)
                    nc.scalar.copy(comb_in[:, bl * 128 + 64:bl * 128 + 96],
                                   hp[:])
                zs = objp.tile([128, NB * 32], F32)
                zs3 = fap(zs, 0, [[32, NB], [1, 32]])
                nc.vector.tensor_tensor(zs3,
                                        fap(osizeO, 0, [[1, NB], [0, 32]]),
                                        fap(szw, 0, [[0, NB], [1, 32]]),
                                        op.mult)
                nc.vector.tensor_tensor(zs3, zs3,
                                        fap(szb, 0, [[0, NB], [1, 32]]),
                                        op.add)
                layernorm_relu(zs, 32, szg, szbb,
                               out_ap=fap(comb_in, 96, [[128, NB], [1, 32]]))
                zc = objp.tile([128, NB * 128], F32)
                for bl in range(NB):
                    ptx = mps.tile([128, 128], F32, tag="tp")
                    nc.tensor.transpose(ptx[:],
                                        comb_in[:, bl * 128:(bl + 1) * 128],
                                        idnf[:])
                    xt = msb.tile([128, 128], F32, tag="xtf")
                    nc.scalar.copy(xt[:], ptx[:])
                    hp = mps.tile([128, 128], F32, tag="mm")
                    nc.tensor.matmul(hp[:], xt[:], wc[:], start=True, stop=True)
                    nc.scalar.copy(zc[:, bl * 128:(bl + 1) * 128], hp[:])
                zc3 = fap(zc, 0, [[128, NB], [1, 128]])
                nc.vector.tensor_tensor(zc3, zc3,
                                        fap(bc, 0, [[0, NB], [1, 128]]),
                                        op.add)
                layernorm_relu(zc, 128, gc, bbc)
                nc.vector.tensor_tensor(zc3, zc3,
                                        fap(vobjO, 0, [[1, NB], [0, 128]]),
                                        op.mult)
                nc.sync.dma_start(
                    bass.AP(feats_o, 0, [[128, 2], [K * 128, NS],
                                         [128 * 2, NB], [1, 128]]),
                    fap(zc, 0, [[128, NB], [1, 128]]))
                vobj8 = objp.tile([128, NB], U8)
                nc.vector.tensor_copy(vobj8[:], vobjO[:])
                nc.sync.dma_start(
                    bass.AP(vobj_o, 0, [[1, 2], [K, NS], [2, NB]]),
                    vobj8[:])
    return nc


# ======================= host driver =======================

def _build_compiled():
    import concourse.bacc as bacc
    nc = bacc.Bacc("TRN2", target_bir_lowering=False, debug=False)
    build_program(nc)
    nc.compile()
    return nc


def kernel(**inputs):
    """Full-input entry: shards batch over 8 NeuronCores, returns full outputs."""
    from concourse.bass_utils import run_bass_kernel_spmd
    x = np.ascontiguousarray(np.asarray(inputs["x"], dtype=np.float32))
    B = x.shape[0]
    n_cores = 8
    per = B // n_cores
    params = {k: np.asarray(v) for k, v in inputs.items() if k != "x"}
    consts = build_consts(params)
    nc = _build_compiled()
    in_maps = []
    for i in range(n_cores):
        m = {"x": np.ascontiguousarray(x[i * per:(i + 1) * per])}
        m.update(consts)
        in_maps.append(m)
    res = run_bass_kernel_spmd(nc, in_maps, list(range(n_cores)))
    feats = np.concatenate([res.results[i]["feats"] for i in range(n_cores)], 0)
    vobj = np.concatenate([res.results[i]["vobj"] for i in range(n_cores)], 0)
    masks = np.concatenate([res.results[i]["masks"] for i in range(n_cores)], 0)
    feats = feats.reshape(B, K, 128).astype(np.float32)
    vobj = vobj.reshape(B, K).astype(bool)
    masks = masks.reshape(B, K, 64, 64).astype(bool)
    return feats, vobj, masks
